# revision 1
# baseline (speedup 1.0000x reference)
"""CenterLoss kernel for Trainium2 (Bass/Tile), data-parallel over 8 NeuronCores.

reference:
    d_i = ||x_i||^2 + ||centers[l_i]||^2 - 2 x_i . centers[l_i]   (= ||x_i - c_{l_i}||^2)
    loss = mean_i clip(d_i, 1e-12, 1e12)

Only the label-gathered entry of the [N, C] distance matrix is used, so the
kernel never forms it: each core gathers centers[labels] with the Q7
dma_gather extended instruction (2048 rows per instruction), computes
(x - c)^2 via DVE subtract + ACT square-with-accumulate, reduces to a scalar
partial sum, and the host combines the 8 partials into the mean.
The clip is a provable no-op for this input distribution (d_i ~ chi^2-like,
concentrated around 256; min over N is >> 1e-12).

Sharding: x/labels split into 8 contiguous row shards; centers replicated.

Layouts per core (ROWS=8192 rows, D=128):
  x tile, chunk c: [128, 16*128] f32, partition p holds rows c*2048 + p*16 .. +15
                   (8 KiB contiguous per partition -> efficient DMA)
  gather, chunk c: dma_gather dst[i%128, i//128, :] = centers[idx_i], so host
                   orders idx_i = labels[c*2048 + (i%128)*16 + (i//128)] to
                   match the x layout. Indices int16, wrapped over 16
                   partitions: idxs[i%16, c*128 + i//16].
"""

import numpy as np

import concourse.bacc as bacc
import concourse.bass as bass
import concourse.tile as tile
from concourse import mybir
from concourse.bass_utils import run_bass_kernel_spmd
from concourse.library_config import mlp

N, C, D = 65536, 1000, 128
N_CORES = 8
P = 128
ROWS_PER_CORE = N // N_CORES            # 8192
CHUNK_ROWS = 512                        # rows gathered/processed per chunk
NCHUNK = ROWS_PER_CORE // CHUNK_ROWS    # 16
SUB = CHUNK_ROWS // P                   # 16 rows per partition per chunk
IDXCOLS = CHUNK_ROWS // 16              # 128 idx columns per chunk

_NC = None


def _build_nc():
    f32 = mybir.dt.float32
    nc = bacc.Bacc(trn_type="TRN2", num_swdge_queues=4, dynamic_dma_scratch_size=65536)

    x = nc.dram_tensor("x", [ROWS_PER_CORE, D], f32, kind="ExternalInput")
    idx16 = nc.dram_tensor(
        "idx16", [P, NCHUNK * IDXCOLS], mybir.dt.int16, kind="ExternalInput"
    )
    centers = nc.dram_tensor("centers", [C, D], f32, kind="ExternalInput")
    out = nc.dram_tensor("out", [1, 1], f32, kind="ExternalOutput")

    # [NCHUNK, P, SUB*D]; partition p of chunk c holds rows c*2048 + p*16 .. +15
    x_r = x.ap().rearrange("(c p s) d -> c p (s d)", p=P, s=SUB)

    with tile.TileContext(nc) as tc:
        with (
            tc.tile_pool(name="xp", bufs=16) as xp,
            tc.tile_pool(name="cp", bufs=16) as cp,
            tc.tile_pool(name="small", bufs=1) as small,
            tc.tile_pool(name="psp", bufs=1, space="PSUM") as psp,
        ):
            # eager Q7 library load so the first gather doesn't stall on the
            # lazy IRAM code fetch
            nc.gpsimd.load_library(mlp)

            idx = small.tile([P, NCHUNK * IDXCOLS], mybir.dt.int16)
            nc.sync.dma_start(out=idx[:], in_=idx16.ap())

            acc = small.tile([P, NCHUNK], f32)
            # queues 1-3 generate descriptors on background Q7 workers; queue 0
            # generates inline on the Pool engine (a 4th worker) while the
            # background queues churn. Small chunks start data drains early.
            # queues 1-3 run on background Q7 workers; queue 0 generates inline
            # on the engine. Each period: 6 background enqueues, then 2 inline
            # gens (workers churn while the engine generates). The period of 8
            # matches the 8 DMASW sem lanes so lanes stay queue-consistent.
            QUEUE = [1, 2, 3, 0] * 4
            xts, cts = {}, {}
            for c in range(NCHUNK):
                xt = xp.tile([P, SUB * D], f32, tag="xt")
                nc.sync.dma_start(out=xt[:], in_=x_r[c])
                ct = cp.tile([P, SUB * D], f32, tag="ct")
                nc.gpsimd.dma_gather(
                    ct[:].rearrange("p (s d) -> p s d", s=SUB),
                    centers.ap(),
                    idx[:, c * IDXCOLS:(c + 1) * IDXCOLS],
                    CHUNK_ROWS,
                    CHUNK_ROWS,
                    D,
                    queue_num=QUEUE[c],
                    single_packet=False,
                )
                xts[c], cts[c] = xt, ct
            for c in range(NCHUNK):
                xt, ct = xts[c], cts[c]
                nc.vector.tensor_tensor(
                    out=xt[:], in0=xt[:], in1=ct[:], op=mybir.AluOpType.subtract
                )
                nc.scalar.activation(
                    out=xt[:],
                    in_=xt[:],
                    func=mybir.ActivationFunctionType.Square,
                    accum_out=acc[:, c:c + 1],
                )

            dsum = small.tile([P, 1], f32)
            nc.vector.tensor_reduce(
                out=dsum[:], in_=acc[:], axis=mybir.AxisListType.X,
                op=mybir.AluOpType.add,
            )
            ones = small.tile([P, 1], f32)
            nc.vector.memset(ones[:], 1.0)
            ps = psp.tile([1, 1], f32)
            nc.tensor.matmul(out=ps[:], lhsT=ones[:], rhs=dsum[:], start=True, stop=True)
            res = small.tile([1, 1], f32)
            nc.vector.tensor_copy(out=res[:], in_=ps[:])
            nc.sync.dma_start(out=out.ap(), in_=res[:])

    nc.compile()
    return nc


def _get_nc():
    global _NC
    if _NC is None:
        _NC = _build_nc()
    return _NC


def _make_idx16(lab_core):
    """Wrap one core's labels into the dma_gather int16 index layout."""
    idx16 = np.zeros((16, NCHUNK * IDXCOLS), dtype=np.int16)
    i = np.arange(CHUNK_ROWS)
    for c in range(NCHUNK):
        vals = lab_core[c * CHUNK_ROWS + (i % P) * SUB + (i // P)]
        idx16[i % 16, c * IDXCOLS + i // 16] = vals.astype(np.int16)
    # the 8 Q7 cores each read their own 16-partition replica of the indices
    return np.ascontiguousarray(np.tile(idx16, (8, 1)))


def make_in_maps(x, labels, centers):
    x = np.ascontiguousarray(np.asarray(x), dtype=np.float32)
    labels_np = np.asarray(labels).astype(np.int64)
    centers = np.ascontiguousarray(np.asarray(centers), dtype=np.float32)
    in_maps = []
    for m in range(N_CORES):
        lo = m * ROWS_PER_CORE
        in_maps.append({
            "x": x[lo:lo + ROWS_PER_CORE],
            "idx16": _make_idx16(labels_np[lo:lo + ROWS_PER_CORE]),
            "centers": centers,
        })
    return in_maps


def run(x, labels, centers, **spmd_kwargs):
    """Run on the 8 NeuronCores; returns (loss, BassKernelResults)."""
    nc = _get_nc()
    in_maps = make_in_maps(x, labels, centers)
    res = run_bass_kernel_spmd(nc, in_maps, core_ids=list(range(N_CORES)), **spmd_kwargs)
    total = sum(float(r["out"][0, 0]) for r in res.results)
    return np.float32(total / N), res


def kernel(x, labels, centers):
    loss, _ = run(x, labels, centers)
    return loss



# revision 5
# speedup vs baseline: 1.0338x; 1.0338x over previous
"""CenterLoss kernel for Trainium2 (Bass/Tile), data-parallel over 8 NeuronCores.

reference:
    d_i = ||x_i||^2 + ||centers[l_i]||^2 - 2 x_i . centers[l_i]
    loss = mean_i clip(d_i, 1e-12, 1e12)
(clip is a provable no-op for this distribution; d_i ~ 256 >> 1e-12.)

Expansion used on device (per core, rows r of its 8192-row shard):
    A_J = sum_{r in superchunk J} ||x_r||^2          (ACT square+accum, 4 chunks)
    B_c = sum_{r in chunk c} x_r . centers[l_r]      (DVE mult+accum on the
                                                      Q7 dma_gather'ed rows)
    C   = sum_c n_c ||c_c||^2                        (n_c = per-core label
                                                      histogram, host-built
                                                      index data)
    loss = sum_cores (sum A - 2 sum B + C) / N       (host combine)

The gather is the wall: SWDGE descriptor generation on the Q7 cores runs at
~10.7ns/row per queue, 4 queues (ucode max) in parallel -> ~22us for 8192
rows.  Everything else (x stream 11.6us on 16 DMA engines, DVE ~11us,
ACT ~10us) hides underneath it.  Keys vs the old version: gathers start
right after the framework preamble (~8us) instead of ~20us, the x stream
is 4 big DMAs with 8KB-contiguous descriptors instead of 16 slow-issued
2KB ones, and the gather-dependent tail is one 0.6us DVE op per chunk.

Layouts per core (8192 rows, D=128):
  x superchunk J (2048 rows): tile [128, 16*128], partition p holds rows
      J*2048 + p*16 .. +15  (8KB contiguous per partition).
  gather chunk c (512 rows) = superchunk J=c//4, quarter q=c%4:
      dma_gather dst[i%128, 4q + i//128, :] = centers[idx_i], so the host
      orders idx_i = labels[J*2048 + (i%128)*16 + 4q + (i//128)].
      Indices int16 wrapped over 16 partitions: idx[i%16, c*32 + i//16],
      replicated x8 so each Q7 core reads its own 16-partition copy.
  centers: HBM padded to [1024, 128] (gather source) and an SBUF copy
      [128, (j d)] (partition p, block j holds center j*128+p) for the
      ||c||^2 / counts term.
"""

import numpy as np

import concourse.bacc as bacc
import concourse.bass as bass
import concourse.tile as tile
from concourse import mybir
from concourse.bass_utils import run_bass_kernel_spmd
from concourse.library_config import mlp

N, C, D = 65536, 1000, 128
CPAD = 1024                              # centers padded to 8*128 rows
N_CORES = 8
P = 128
ROWS_PER_CORE = N // N_CORES             # 8192
CHUNK = 512                              # rows per gather instruction
NCHUNK = ROWS_PER_CORE // CHUNK          # 16
SUPER = 2048                             # rows per x DMA
NSUPER = ROWS_PER_CORE // SUPER          # 4
QPERS = SUPER // CHUNK                   # 4 gather chunks per superchunk
IDXC = CHUNK // 16                       # 32 idx cols per chunk
NOUT = NCHUNK + NSUPER + 1               # 21 partial sums out

_NC = None


def _build_nc():
    f32 = mybir.dt.float32
    i16 = mybir.dt.int16
    nc = bacc.Bacc(trn_type="TRN2", num_swdge_queues=4, dynamic_dma_scratch_size=65536)

    x = nc.dram_tensor("x", [ROWS_PER_CORE, D], f32, kind="ExternalInput")
    idx16 = nc.dram_tensor("idx16", [P, NCHUNK * IDXC], i16, kind="ExternalInput")
    centers = nc.dram_tensor("centers", [CPAD, D], f32, kind="ExternalInput")
    counts = nc.dram_tensor("counts", [P, CPAD // P], f32, kind="ExternalInput")
    out = nc.dram_tensor("out", [1, NOUT], f32, kind="ExternalOutput")

    # [NSUPER, P, 16*D]; partition p of superchunk J holds rows J*2048+p*16..+15
    x_r = x.ap().rearrange("(J p s) d -> J p (s d)", p=P, s=SUPER // P)
    # centers SBUF copy: partition p, block j <- center j*128+p
    ctr_r = centers.ap().rearrange("(j p) d -> p j d", p=P)

    with tile.TileContext(nc) as tc:
        with (
            tc.tile_pool(name="xp", bufs=NSUPER) as xp,
            tc.tile_pool(name="cp", bufs=NSUPER) as cp,
            tc.tile_pool(name="scr", bufs=2) as scr,
            tc.tile_pool(name="small", bufs=1) as small,
            tc.tile_pool(name="psp", bufs=1, space="PSUM") as psp,
        ):
            # eager Q7 library load so the first gather doesn't stall on IRAM
            nc.gpsimd.load_library(mlp)

            # chunk-0 indices land first (8KB) so gather 0 starts ASAP
            idxA = small.tile([P, IDXC], i16)
            nc.sync.dma_start(out=idxA[:], in_=idx16.ap()[:, 0:IDXC])
            idxB = small.tile([P, (NCHUNK - 1) * IDXC], i16)
            nc.sync.dma_start(out=idxB[:], in_=idx16.ap()[:, IDXC:])

            ctr = small.tile([P, CPAD // P * D], f32)
            nc.sync.dma_start(
                out=ctr[:].rearrange("p (j d) -> p j d", j=CPAD // P), in_=ctr_r
            )
            cnt = small.tile([P, CPAD // P], f32)
            nc.sync.dma_start(out=cnt[:], in_=counts.ap())

            xts = []
            for J in range(NSUPER):
                xt = xp.tile([P, SUPER // P * D], f32, tag="xt")
                nc.sync.dma_start(out=xt[:], in_=x_r[J])
                xts.append(xt)

            accB = small.tile([P, NCHUNK], f32)
            accA = small.tile([P, NSUPER], f32)
            accC = small.tile([P, 1], f32)
            sq8 = small.tile([P, CPAD // P], f32)
            ones = small.tile([P, 1], f32)

            # ---- gathers: first thing on the Q7 queues after lib load ----
            QUEUE = [1, 2, 3, 0] * (NCHUNK // 4)
            cts = []
            for J in range(NSUPER):
                ct = cp.tile([P, SUPER // P * D], f32, tag="ct")
                cts.append(ct)
            for c in range(NCHUNK):
                J, q = c // QPERS, c % QPERS
                ct3 = cts[J][:].rearrange("p (s d) -> p s d", s=SUPER // P)
                idx_sl = idxA[:, 0:IDXC] if c == 0 else idxB[:, (c - 1) * IDXC:c * IDXC]
                nc.gpsimd.dma_gather(
                    ct3[:, 4 * q:4 * q + 4, :],
                    centers.ap(),
                    idx_sl,
                    CHUNK,
                    CHUNK,
                    D,
                    queue_num=QUEUE[c],
                    single_packet=False,
                )

            # ---- DVE: ones, ||c||^2, C term, then per-chunk x.c accum ----
            nc.vector.memset(ones[:], 1.0)
            for j in range(CPAD // P):
                sl = ctr[:, j * D:(j + 1) * D]
                nc.vector.scalar_tensor_tensor(
                    out=sl, in0=sl, scalar=1.0, in1=sl,
                    op0=mybir.AluOpType.mult, op1=mybir.AluOpType.mult,
                    accum_out=sq8[:, j:j + 1],
                )
            nc.vector.scalar_tensor_tensor(
                out=cnt[:], in0=sq8[:], scalar=1.0, in1=cnt[:],
                op0=mybir.AluOpType.mult, op1=mybir.AluOpType.mult,
                accum_out=accC[:],
            )
            for c in range(NCHUNK):
                J, q = c // QPERS, c % QPERS
                xs = xts[J][:, q * CHUNK:(q + 1) * CHUNK]
                cs = cts[J][:, q * CHUNK:(q + 1) * CHUNK]
                nc.vector.scalar_tensor_tensor(
                    out=cs, in0=xs, scalar=1.0, in1=cs,
                    op0=mybir.AluOpType.mult, op1=mybir.AluOpType.mult,
                    accum_out=accB[:, c:c + 1],
                )

            # ---- ACT: ||x||^2 per superchunk ----
            for J in range(NSUPER):
                sA = scr.tile([P, SUPER // P * D], f32, tag="sA")
                nc.scalar.activation(
                    out=sA[:], in_=xts[J][:],
                    func=mybir.ActivationFunctionType.Square,
                    accum_out=accA[:, J:J + 1],
                )

            # ---- reduce partials across partitions and write out ----
            ps = psp.tile([1, NOUT], f32)
            nc.tensor.matmul(out=ps[:, 0:NCHUNK], lhsT=ones[:], rhs=accB[:],
                             start=True, stop=True)
            nc.tensor.matmul(out=ps[:, NCHUNK:NCHUNK + NSUPER], lhsT=ones[:],
                             rhs=accA[:], start=True, stop=True)
            nc.tensor.matmul(out=ps[:, NOUT - 1:NOUT], lhsT=ones[:], rhs=accC[:],
                             start=True, stop=True)
            res = small.tile([1, NOUT], f32)
            nc.vector.tensor_copy(out=res[:], in_=ps[:])
            nc.sync.dma_start(out=out.ap(), in_=res[:])

    nc.compile()
    return nc


def _get_nc():
    global _NC
    if _NC is None:
        _NC = _build_nc()
    return _NC


def _make_idx16(lab_core):
    """Wrap one core's labels into the dma_gather int16 index layout."""
    i = np.arange(CHUNK)
    idx16 = np.zeros((16, NCHUNK * IDXC), dtype=np.int16)
    for c in range(NCHUNK):
        J, q = c // QPERS, c % QPERS
        rows = J * SUPER + (i % P) * (SUPER // P) + 4 * q + (i // P)
        idx16[i % 16, c * IDXC + i // 16] = lab_core[rows].astype(np.int16)
    # each of the 8 Q7 cores reads its own 16-partition replica
    return np.ascontiguousarray(np.tile(idx16, (8, 1)))


def make_in_maps(x, labels, centers):
    x = np.ascontiguousarray(np.asarray(x), dtype=np.float32)
    labels_np = np.asarray(labels).astype(np.int64)
    centers = np.ascontiguousarray(np.asarray(centers), dtype=np.float32)
    cpad = np.zeros((CPAD, D), dtype=np.float32)
    cpad[:C] = centers
    in_maps = []
    for m in range(N_CORES):
        lab = labels_np[m * ROWS_PER_CORE:(m + 1) * ROWS_PER_CORE]
        cnt = np.bincount(lab, minlength=CPAD).astype(np.float32)
        in_maps.append({
            "x": x[m * ROWS_PER_CORE:(m + 1) * ROWS_PER_CORE],
            "idx16": _make_idx16(lab),
            "centers": cpad,
            "counts": np.ascontiguousarray(cnt.reshape(CPAD // P, P).T),
        })
    return in_maps


def run(x, labels, centers, **spmd_kwargs):
    """Run on the 8 NeuronCores; returns (loss, BassKernelResults)."""
    nc = _get_nc()
    in_maps = make_in_maps(x, labels, centers)
    res = run_bass_kernel_spmd(nc, in_maps, core_ids=list(range(N_CORES)), **spmd_kwargs)
    total = 0.0
    for r in res.results:
        o = np.asarray(r["out"], dtype=np.float64)[0]
        total += float(o[NCHUNK:NCHUNK + NSUPER].sum() + o[NOUT - 1]
                       - 2.0 * o[0:NCHUNK].sum())
    return np.float32(total / N), res


def kernel(x, labels, centers):
    loss, _ = run(x, labels, centers)
    return loss


# revision 11
# speedup vs baseline: 1.1150x; 1.0786x over previous
"""CenterLoss kernel for Trainium2 (Bass/Tile), data-parallel over 8 NeuronCores.

reference:
    d_i = ||x_i||^2 + ||centers[l_i]||^2 - 2 x_i . centers[l_i]
    loss = mean_i clip(d_i, 1e-12, 1e12)
(clip is a provable no-op for this distribution; d_i ~ 256 >> 1e-12.)

Expansion used on device (per core, rows r of its 8192-row shard):
    A_J = sum_{r in superchunk J} ||x_r||^2          (ACT square+accum, 4 chunks)
    B_c = sum_{r in chunk c} x_r . centers[l_r]      (DVE mult+accum on the
                                                      Q7 dma_gather'ed rows)
    C   = sum_c n_c ||c_c||^2                        (n_c = per-core label
                                                      histogram, host-built
                                                      index data)
    loss = sum_cores (sum A - 2 sum B + C) / N       (host combine)

The gather is the wall: SWDGE descriptor generation on the Q7 cores runs at
~10.7ns/row per queue, 4 queues (ucode max) in parallel -> ~22us for 8192
rows.  Everything else (x stream 11.6us on 16 DMA engines, DVE ~11us,
ACT ~10us) hides underneath it.  Keys vs the old version: gathers start
right after the framework preamble (~8us) instead of ~20us, the x stream
is 4 big DMAs with 8KB-contiguous descriptors instead of 16 slow-issued
2KB ones, and the gather-dependent tail is one 0.6us DVE op per chunk.

Layouts per core (8192 rows, D=128):
  x superchunk J (2048 rows): tile [128, 16*128], partition p holds rows
      J*2048 + p*16 .. +15  (8KB contiguous per partition).
  gather chunk c (512 rows) = superchunk J=c//4, quarter q=c%4:
      dma_gather dst[i%128, 4q + i//128, :] = centers[idx_i], so the host
      orders idx_i = labels[J*2048 + (i%128)*16 + 4q + (i//128)].
      Indices int16 wrapped over 16 partitions: idx[i%16, c*32 + i//16],
      replicated x8 so each Q7 core reads its own 16-partition copy.
  centers: HBM padded to [1024, 128] (gather source) and an SBUF copy
      [128, (j d)] (partition p, block j holds center j*128+p) for the
      ||c||^2 / counts term.
"""

import numpy as np

import concourse.bacc as bacc
import concourse.bass as bass
import concourse.tile as tile
from concourse import mybir
from concourse.bass_utils import run_bass_kernel_spmd
from concourse.library_config import mlp

N, C, D = 65536, 1000, 128
CPAD = 1024                              # centers padded to 8*128 rows
N_CORES = 8
P = 128
ROWS_PER_CORE = N // N_CORES             # 8192
CHUNK = 512                              # rows per gather instruction
NCHUNK = ROWS_PER_CORE // CHUNK          # 16
SUPER = 2048                             # rows per x DMA
NSUPER = ROWS_PER_CORE // SUPER          # 4
QPERS = SUPER // CHUNK                   # 4 gather chunks per superchunk
IDXC = CHUNK // 16                       # 32 idx cols per chunk
NOUT = 2 * NSUPER + 1                    # 9 partial sums out (B x4, A x4, C)

_NC = None


def _build_nc():
    f32 = mybir.dt.float32
    i16 = mybir.dt.int16
    nc = bacc.Bacc(trn_type="TRN2", num_swdge_queues=4, dynamic_dma_scratch_size=65536)

    x = nc.dram_tensor("x", [ROWS_PER_CORE, D], f32, kind="ExternalInput")
    idx16 = nc.dram_tensor("idx16", [P, NCHUNK * IDXC], i16, kind="ExternalInput")
    centers = nc.dram_tensor("centers", [CPAD, D], f32, kind="ExternalInput")
    counts = nc.dram_tensor("counts", [P, CPAD // P], f32, kind="ExternalInput")
    out = nc.dram_tensor("out", [1, NOUT], f32, kind="ExternalOutput")

    # [NSUPER, P, 16*D]; partition p of superchunk J holds rows J*2048+p*16..+15
    x_r = x.ap().rearrange("(J p s) d -> J p (s d)", p=P, s=SUPER // P)
    # centers SBUF copy: partition p, block j <- center j*128+p
    ctr_r = centers.ap().rearrange("(j p) d -> p j d", p=P)

    with tile.TileContext(nc) as tc:
        with (
            tc.tile_pool(name="xp", bufs=NSUPER) as xp,
            tc.tile_pool(name="cp", bufs=NSUPER) as cp,
            tc.tile_pool(name="scr", bufs=2) as scr,
            tc.tile_pool(name="small", bufs=1) as small,
            tc.tile_pool(name="psp", bufs=1, space="PSUM") as psp,
        ):
            # eager Q7 library load so the first gather doesn't stall on IRAM
            nc.gpsimd.load_library(mlp)

            # chunk-0 indices land first (8KB) so gather 0 starts ASAP
            idxA = small.tile([P, IDXC], i16)
            nc.sync.dma_start(out=idxA[:], in_=idx16.ap()[:, 0:IDXC])
            idxB = small.tile([P, (NCHUNK - 1) * IDXC], i16)
            nc.sync.dma_start(out=idxB[:], in_=idx16.ap()[:, IDXC:])

            ctr = small.tile([P, CPAD // P * D], f32)
            nc.sync.dma_start(
                out=ctr[:].rearrange("p (j d) -> p j d", j=CPAD // P), in_=ctr_r
            )
            cnt = small.tile([P, CPAD // P], f32)
            nc.sync.dma_start(out=cnt[:], in_=counts.ap())

            xts = []
            for J in range(NSUPER):
                xt = xp.tile([P, SUPER // P * D], f32, tag="xt")
                nc.sync.dma_start(out=xt[:], in_=x_r[J])
                xts.append(xt)

            accB = small.tile([P, NSUPER], f32)
            accA = small.tile([P, NSUPER], f32)
            accC = small.tile([P, 1], f32)
            sq8 = small.tile([P, CPAD // P], f32)
            ones = small.tile([P, 1], f32)

            # ---- gathers: first thing on the Q7 queues after lib load ----
            # one shared num_idxs register: a MOVE per gather costs ~500ns of
            # GpSimd SEQ time each and delays the first gather by ~8us
            nreg = nc.gpsimd.to_reg(CHUNK)
            QUEUE = [1, 2, 3, 0] * (NCHUNK // 4)
            cts = []
            for J in range(NSUPER):
                ct = cp.tile([P, SUPER // P * D], f32, tag="ct")
                cts.append(ct)
            for c in range(NCHUNK):
                J, q = c // QPERS, c % QPERS
                ct3 = cts[J][:].rearrange("p (s d) -> p s d", s=SUPER // P)
                idx_sl = idxA[:, 0:IDXC] if c == 0 else idxB[:, (c - 1) * IDXC:c * IDXC]
                nc.gpsimd.dma_gather(
                    ct3[:, 4 * q:4 * q + 4, :],
                    centers.ap(),
                    idx_sl,
                    CHUNK,
                    nreg,
                    D,
                    queue_num=QUEUE[c],
                    single_packet=False,
                )

            # ---- DVE: ones, ||c||^2, C term, then per-chunk x.c accum ----
            nc.vector.memset(ones[:], 1.0)
            for j in range(CPAD // P):
                sl = ctr[:, j * D:(j + 1) * D]
                nc.vector.scalar_tensor_tensor(
                    out=sl, in0=sl, scalar=1.0, in1=sl,
                    op0=mybir.AluOpType.mult, op1=mybir.AluOpType.mult,
                    accum_out=sq8[:, j:j + 1],
                )
            nc.vector.scalar_tensor_tensor(
                out=cnt[:], in0=sq8[:], scalar=1.0, in1=cnt[:],
                op0=mybir.AluOpType.mult, op1=mybir.AluOpType.mult,
                accum_out=accC[:],
            )
            # one x.c accum per superchunk: round J of gathers fills cts[J]
            # entirely, and a [128, 2048] stt has less overhead + shorter
            # tail than four [128, 512] ones
            for J in range(NSUPER):
                nc.vector.scalar_tensor_tensor(
                    out=cts[J][:], in0=xts[J][:], scalar=1.0, in1=cts[J][:],
                    op0=mybir.AluOpType.mult, op1=mybir.AluOpType.mult,
                    accum_out=accB[:, J:J + 1],
                )

            # ---- ACT: ||x||^2 per superchunk ----
            for J in range(NSUPER):
                sA = scr.tile([P, SUPER // P * D], f32, tag="sA")
                nc.scalar.activation(
                    out=sA[:], in_=xts[J][:],
                    func=mybir.ActivationFunctionType.Square,
                    accum_out=accA[:, J:J + 1],
                )

            # ---- reduce partials across partitions and write out ----
            ps = psp.tile([1, NOUT], f32)
            nc.tensor.matmul(out=ps[:, 0:NSUPER], lhsT=ones[:], rhs=accB[:],
                             start=True, stop=True)
            nc.tensor.matmul(out=ps[:, NSUPER:2 * NSUPER], lhsT=ones[:],
                             rhs=accA[:], start=True, stop=True)
            nc.tensor.matmul(out=ps[:, NOUT - 1:NOUT], lhsT=ones[:], rhs=accC[:],
                             start=True, stop=True)
            res = small.tile([1, NOUT], f32)
            nc.vector.tensor_copy(out=res[:], in_=ps[:])
            nc.sync.dma_start(out=out.ap(), in_=res[:])

    nc.compile()
    return nc


def _get_nc():
    global _NC
    if _NC is None:
        _NC = _build_nc()
    return _NC


def _make_idx16(lab_core):
    """Wrap one core's labels into the dma_gather int16 index layout."""
    i = np.arange(CHUNK)
    idx16 = np.zeros((16, NCHUNK * IDXC), dtype=np.int16)
    for c in range(NCHUNK):
        J, q = c // QPERS, c % QPERS
        rows = J * SUPER + (i % P) * (SUPER // P) + 4 * q + (i // P)
        idx16[i % 16, c * IDXC + i // 16] = lab_core[rows].astype(np.int16)
    # each of the 8 Q7 cores reads its own 16-partition replica
    return np.ascontiguousarray(np.tile(idx16, (8, 1)))


def make_in_maps(x, labels, centers):
    x = np.ascontiguousarray(np.asarray(x), dtype=np.float32)
    labels_np = np.asarray(labels).astype(np.int64)
    centers = np.ascontiguousarray(np.asarray(centers), dtype=np.float32)
    cpad = np.zeros((CPAD, D), dtype=np.float32)
    cpad[:C] = centers
    in_maps = []
    for m in range(N_CORES):
        lab = labels_np[m * ROWS_PER_CORE:(m + 1) * ROWS_PER_CORE]
        cnt = np.bincount(lab, minlength=CPAD).astype(np.float32)
        in_maps.append({
            "x": x[m * ROWS_PER_CORE:(m + 1) * ROWS_PER_CORE],
            "idx16": _make_idx16(lab),
            "centers": cpad,
            "counts": np.ascontiguousarray(cnt.reshape(CPAD // P, P).T),
        })
    return in_maps


def run(x, labels, centers, **spmd_kwargs):
    """Run on the 8 NeuronCores; returns (loss, BassKernelResults)."""
    nc = _get_nc()
    in_maps = make_in_maps(x, labels, centers)
    res = run_bass_kernel_spmd(nc, in_maps, core_ids=list(range(N_CORES)), **spmd_kwargs)
    total = 0.0
    for r in res.results:
        o = np.asarray(r["out"], dtype=np.float64)[0]
        total += float(o[NSUPER:2 * NSUPER].sum() + o[NOUT - 1]
                       - 2.0 * o[0:NSUPER].sum())
    return np.float32(total / N), res


def kernel(x, labels, centers):
    loss, _ = run(x, labels, centers)
    return loss
